# revision 1
# baseline (speedup 1.0000x reference)
"""Multi-head self-attention (B=4, S=2048, D=1024, H=16) on 8 NeuronCores.

Sharding: tensor-parallel over heads. Core c owns heads {2c, 2c+1} = 128
columns of Wq/Wk/Wv and 128 rows of Wo. Each core computes Q^T/K^T/V for its
two heads over all tokens, runs attention for its 8 (batch, head) pairs, and
produces a partial output O_c = A_c @ Wo_c.  The all-reduce over the 8
partials is done on the host during unsharding.

On-chip layout (per batch b of 2048 tokens):
  - QKV^T tiles [128, 2048] (head-dim on partitions) from X^T resident chunks
  - V is PE-transposed back to token-major and augmented with a ones column,
    so the attention matmul accumulates both U^T = V^T P and the softmax
    denominators in one PSUM tile (row 64).
  - scores are computed transposed (S^T = K Q^T) with the two heads packed
    into disjoint PE row groups; one fused exp over both heads' PSUM banks.
  - no max-subtraction: scores ~ N(0,1) after the 1/sqrt(d) scale, |s| < ~7.
"""
import os
import sys

for _p in ("/opt/trn_rl_repo", "/root/.axon_site/_ro/trn_rl_repo"):
    if os.path.isdir(_p) and _p not in sys.path:
        sys.path.append(_p)

from contextlib import ExitStack

import numpy as np
import ml_dtypes

import concourse.bass as bass
import concourse.tile as tile
from concourse import mybir
from concourse.bass_utils import run_bass_kernel_spmd
from concourse.masks import make_identity

BF16 = mybir.dt.bfloat16
F32 = mybir.dt.float32
EXP = mybir.ActivationFunctionType.Exp
NP_BF16 = ml_dtypes.bfloat16

B, S, D = 4, 2048, 1024
H, HD = 16, 64
N_CORES = 8
T = B * S  # 8192 tokens
KC = D // 128  # 8 contraction chunks
SCALE = 1.0 / np.sqrt(HD)

# ---------------------------------------------------------------------------
# Tile patches: this walrus build rejects instructions with more than one
# sync wait ("Too many sync wait commands"), so split extra waits into
# preceding same-engine nops, and replace the kernel-tail drain's wait list
# with a chain of single-wait SP nops.
# ---------------------------------------------------------------------------
_MAX_WAITS = 1
_patched = False


def _install_tile_patches():
    global _patched
    if _patched:
        return
    _patched = True
    from concourse.vector_clock import ScopedClock, VectorClock

    orig_lower = tile.TileContext._lower_ordered_insts

    def split_inst_waits(self, ordered):
        for bb_name in list(ordered.keys()):
            insts = ordered[bb_name]
            new = []
            for inst in insts:
                si = inst.sync_info
                if si is not None and len(si.on_wait) > _MAX_WAITS:
                    waits = list(si.on_wait)
                    head, tail = waits[:-_MAX_WAITS], waits[-_MAX_WAITS:]
                    for w in head:
                        nop = mybir.InstNoOp(
                            name=f"ws-{self.nc.next_id()}",
                            engine=inst.engine,
                            bass_nofuse=True,
                        )
                        nop.sync_info = mybir.SyncInfo(on_wait=[w], on_update=[])
                        new.append(nop)
                    inst.sync_info = mybir.SyncInfo(
                        on_wait=tail, on_update=list(si.on_update)
                    )
                new.append(inst)
            ordered[bb_name] = new
        return orig_lower(self, ordered)

    def split_drain_and_barrier(self, tick_clock, wait_clock):
        gc = tick_clock.global_clock
        ticks = eval(repr(gc).replace("VectorClock", ""))
        procs = [(i, t) for i, t in enumerate(ticks) if t > 0]
        for i in range(0, len(procs), _MAX_WAITS):
            chunk = procs[i : i + _MAX_WAITS]
            nop = self.nc.sync.nop(nofuse=True, hint="drain_wait_split")
            pc = VectorClock()
            for proc, tick in chunk:
                pc.require_at_least(proc, tick)
            wait_clock.add_sem_waits(nop.ins, ScopedClock({None: pc}))
        drain_inst = self.nc.sync.drain()
        wait_clock.add_sem_waits(
            drain_inst.ins, ScopedClock({None: gc}), ScopedClock({None: gc.copy()})
        )
        self.nc.all_engine_barrier()
        assert self.sems is not None
        popped = self.nc._tile_sem_poison_stack.pop()
        assert popped is self._sem_poison
        self.nc.clear_and_free_semaphores(list(self.sems.allocated().values()))
        self.nc.all_engine_barrier()

    tile.TileContext._lower_ordered_insts = split_inst_waits
    tile.TileContext._drain_and_barrier = split_drain_and_barrier


# ---------------------------------------------------------------------------
# Device kernel
# ---------------------------------------------------------------------------
def build_attention_nc(with_bias=True, probe=None, out_bf16=False):
    _install_tile_patches()
    nc = bass.Bass()

    xT = nc.declare_dram_parameter("xT", [KC, 128, T], BF16, isOutput=False)
    wq = nc.declare_dram_parameter("wq", [KC, 128, 128], BF16, isOutput=False)
    wk = nc.declare_dram_parameter("wk", [KC, 128, 128], BF16, isOutput=False)
    wv = nc.declare_dram_parameter("wv", [KC, 128, 128], BF16, isOutput=False)
    if with_bias:
        bq = nc.declare_dram_parameter("bq", [128], BF16, isOutput=False)
        bk = nc.declare_dram_parameter("bk", [128], BF16, isOutput=False)
        bv = nc.declare_dram_parameter("bv", [128], BF16, isOutput=False)
    else:
        bq = bk = bv = None
    wo = nc.declare_dram_parameter("wo", [128, D], BF16, isOutput=False)
    out = nc.declare_dram_parameter(
        "out", [T, D], BF16 if out_bf16 else F32, isOutput=True
    )

    with tile.TileContext(nc) as tc, ExitStack() as ctx:
        singles = ctx.enter_context(tc.tile_pool(name="singles", bufs=1))
        px = ctx.enter_context(tc.tile_pool(name="px", bufs=16))
        pqk = ctx.enter_context(tc.tile_pool(name="pqk", bufs=2))
        pv = ctx.enter_context(tc.tile_pool(name="pv", bufs=2))
        pa = ctx.enter_context(tc.tile_pool(name="pa", bufs=2))
        ppt = ctx.enter_context(tc.tile_pool(name="ppt", bufs=4))
        pnorm = ctx.enter_context(tc.tile_pool(name="pnorm", bufs=4))
        pob = ctx.enter_context(tc.tile_pool(name="pob", bufs=3))
        dsc = ctx.enter_context(tc.tile_pool(name="dsc", bufs=8, space="DRAM"))
        psA = ctx.enter_context(tc.tile_pool(name="psA", bufs=2, space="PSUM"))
        psS = ctx.enter_context(tc.tile_pool(name="psS", bufs=2, space="PSUM"))
        psU = ctx.enter_context(tc.tile_pool(name="psU", bufs=2, space="PSUM"))

        # --- constants / weights, loaded once -----------------------------
        w_sb = {}
        b_sb = {}
        for name, wd, bd in (("q", wq, bq), ("k", wk, bk), ("v", wv, bv)):
            w_t = singles.tile([128, KC, 128], BF16, tag=f"w{name}")
            nc.sync.dma_start(w_t, wd[:, :, :].rearrange("k p m -> p k m"))
            w_sb[name] = w_t
            if with_bias:
                b_t = singles.tile([1, 128], BF16, tag=f"b{name}")
                nc.sync.dma_start(b_t, bd[:][None, :])
                b_sb[name] = b_t
        wo_sb = singles.tile([128, D], BF16, tag="wo")
        nc.sync.dma_start(wo_sb, wo[:, :])
        if with_bias:
            ones_sb = singles.tile([1, 512], BF16, tag="ones")
            nc.vector.memset(ones_sb, 1.0)
        ident = singles.tile([128, 128], BF16, tag="ident")
        make_identity(nc, ident)

        # per-batch state created by the A-slices
        state = [dict() for _ in range(B)]

        def a_slice(b, qt):
            """Emit 1/4 of batch b's QKV projections (+ V transposes)."""
            st = state[b]
            if qt == 0:
                st["x"] = [
                    px.tile([128, S], BF16, tag="x", name=f"x_{b}_{kc}")
                    for kc in range(KC)
                ]
                if probe != "nox":
                    for kc in range(KC):
                        nc.sync.dma_start(
                            st["x"][kc], xT[kc, :, b * S : (b + 1) * S]
                        )
                for name in ("q", "k", "v"):
                    pool = pqk if name != "v" else pv
                    st[name] = pool.tile([128, S], BF16, tag=f"{name}T", name=f"{name}T_{b}")
                st["vS0"] = pv.tile([128, 16, 65], BF16, tag="vS0", name=f"vS0_{b}")
                st["vS1"] = pv.tile([128, 16, 65], BF16, tag="vS1", name=f"vS1_{b}")
                nc.vector.memset(st["vS0"][:, :, 64:65], 1.0)
                nc.vector.memset(st["vS1"][:, :, 64:65], 1.0)
            for name in ("q", "k", "v"):
                dst = st[name]
                w_t, b_t = w_sb[name], b_sb.get(name)
                ps = psA.tile([128, 512], F32, tag="psA")
                for kc in range(KC):
                    nc.tensor.matmul(
                        ps,
                        w_t[:, kc, :],
                        st["x"][kc][:, qt * 512 : (qt + 1) * 512],
                        start=(kc == 0),
                        stop=(not with_bias and kc == KC - 1),
                    )
                if with_bias:
                    nc.tensor.matmul(ps, b_t, ones_sb, start=False, stop=True)
                nc.vector.tensor_copy(dst[:, qt * 512 : (qt + 1) * 512], ps)
            # transpose this quarter of V into token-major + ones layout
            for t in range(qt * 4, qt * 4 + 4):
                tp = psU.tile([128, 128], BF16, tag="u")
                nc.tensor.transpose(tp, st["v"][:, t * 128 : (t + 1) * 128], ident)
                nc.vector.tensor_copy(st["vS0"][:, t, 0:64], tp[:, 0:64])
                nc.vector.tensor_copy(st["vS1"][:, t, 0:64], tp[:, 64:128])

        def b_block(b, qt):
            """scores^T -> exp -> attention -> normalize, for one q-tile."""
            st = state[b]
            qT, kT = st["q"], st["k"]
            q0, q1 = qt * 512, (qt + 1) * 512
            u0 = psU.tile([128, 512], F32, tag="u")
            u1 = psU.tile([128, 512], F32, tag="u")
            for kc in range(16):
                k0 = kc * 128
                sp = psS.tile([128, 1024], F32, tag="psS")
                nc.tensor.matmul(
                    sp[:, 0:512], kT[0:64, k0 : k0 + 128], qT[0:64, q0:q1],
                    start=True, stop=True, tile_position=(0, 0),
                )
                if probe == "nopack":
                    nc.tensor.matmul(
                        sp[:, 512:1024], kT[0:64, k0 : k0 + 128], qT[0:64, q0:q1],
                        start=True, stop=True, tile_position=(0, 0),
                    )
                else:
                    nc.tensor.matmul(
                        sp[:, 512:1024], kT[64:128, k0 : k0 + 128], qT[64:128, q0:q1],
                        start=True, stop=True, tile_position=(64, 0),
                    )
                pt = ppt.tile([128, 1024], BF16, tag="pt")
                if probe == "noact":
                    nc.vector.memset(pt, 0.001)
                else:
                    nc.scalar.activation(pt, sp, EXP, scale=float(SCALE))
                nc.tensor.matmul(
                    u0[0:65, :], st["vS0"][:, kc, :], pt[:, 0:512],
                    start=(kc == 0), stop=(kc == 15),
                )
                nc.tensor.matmul(
                    u1[0:65, :], st["vS1"][:, kc, :], pt[:, 512:1024],
                    start=(kc == 0), stop=(kc == 15),
                )
            aTq = pa.tile([128, 512], BF16, tag="aT", name=f"aT_{b}_{qt}")
            st[f"aT{qt}"] = aTq
            for h, u in ((0, u0), (1, u1)):
                # copy out of PSUM right away so the u slot frees for the
                # next q-tile; the norm chain continues on SBUF tiles.
                usb = pnorm.tile([65, 512], F32, tag="usb")
                nc.vector.tensor_copy(usb, u[0:65, :])
                dinv = pnorm.tile([1, 512], F32, tag="dinv")
                nc.vector.reciprocal(dinv, usb[64:65, :])
                sc = dsc.tile([1, 512], F32, tag="sc")
                nc.sync.dma_start(sc, dinv)
                bc = pnorm.tile([64, 512], F32, tag="bc")
                nc.sync.dma_start(bc, sc.to_broadcast((64, 512)))
                nc.vector.tensor_mul(
                    aTq[h * 64 : (h + 1) * 64, :], usb[0:64, :], bc
                )

        def c_slice(b, j):
            """Emit 4 output-projection token tiles for batch b (tt=4j..4j+3)."""
            st = state[b]
            aTq = st[f"aT{j}"]
            for tt in range(4 * j, 4 * j + 4):
                col = (tt - 4 * j) * 128
                ob = pob.tile([128, 1024], BF16 if out_bf16 else F32, tag="ob")
                for g in range(2):
                    po = psA.tile([128, 512], F32, tag="psA")
                    nc.tensor.matmul(
                        po,
                        aTq[:, col : col + 128],
                        wo_sb[:, g * 512 : (g + 1) * 512],
                        start=True,
                        stop=True,
                    )
                    nc.vector.tensor_copy(ob[:, g * 512 : (g + 1) * 512], po)
                if probe != "noout" or tt == 0:
                    nc.sync.dma_start(
                        out[b * S + tt * 128 : b * S + (tt + 1) * 128, :], ob
                    )

        # software pipeline: A(0) primed; then per batch, B(b,qt) leads while
        # A(b+1) and C(b) slices fill engine gaps.
        for qt in range(4):
            a_slice(0, qt)
        for b in range(B):
            for qt in range(4):
                b_block(b, qt)
                if b + 1 < B:
                    a_slice(b + 1, qt)
                if qt >= 1:
                    c_slice(b, qt - 1)
            c_slice(b, 3)

    return nc


_NC_CACHE = {}


def _get_nc(with_bias=True, probe=None, out_bf16=False):
    key = (with_bias, probe, out_bf16)
    if key not in _NC_CACHE:
        _NC_CACHE[key] = build_attention_nc(with_bias, probe, out_bf16)
    return _NC_CACHE[key]


def _run(inputs, Wq, bq, Wk, bk, Wv, bv, Wo, bo, trace=False, **spmd_kwargs):
    X2 = np.asarray(inputs, dtype=np.float32).reshape(T, D)
    xT = X2.T.astype(NP_BF16).reshape(KC, 128, T)
    with_bias = bool(
        np.any(np.asarray(bq)) or np.any(np.asarray(bk)) or np.any(np.asarray(bv))
    )

    in_maps = []
    for c in range(N_CORES):
        cs = slice(c * 128, (c + 1) * 128)
        in_maps.append(
            {
                "xT": xT,
                "wq": np.ascontiguousarray(Wq[:, cs]).astype(NP_BF16).reshape(KC, 128, 128),
                "wk": np.ascontiguousarray(Wk[:, cs]).astype(NP_BF16).reshape(KC, 128, 128),
                "wv": np.ascontiguousarray(Wv[:, cs]).astype(NP_BF16).reshape(KC, 128, 128),
                "bq": np.asarray(bq[cs]).astype(NP_BF16),
                "bk": np.asarray(bk[cs]).astype(NP_BF16),
                "bv": np.asarray(bv[cs]).astype(NP_BF16),
                "wo": np.ascontiguousarray(Wo[cs, :]).astype(NP_BF16),
            }
        )

    if not with_bias:
        for m in in_maps:
            m.pop("bq"), m.pop("bk"), m.pop("bv")
    res = run_bass_kernel_spmd(
        _get_nc(with_bias), in_maps, list(range(N_CORES)), trace=trace, **spmd_kwargs
    )
    acc = res.results[0]["out"].astype(np.float32)
    for c in range(1, N_CORES):
        acc += res.results[c]["out"]
    acc += np.asarray(bo, dtype=np.float32)[None, :]
    return acc.reshape(B, S, D), res


def kernel(inputs, Wq, bq, Wk, bk, Wv, bv, Wo, bo):
    out, _ = _run(inputs, Wq, bq, Wk, bk, Wv, bv, Wo, bo)
    return out



# revision 2
# speedup vs baseline: 1.2730x; 1.2730x over previous
"""Multi-head self-attention (B=4, S=2048, D=1024, H=16) on 8 NeuronCores.

Sharding: tensor-parallel over heads. Core c owns heads {2c, 2c+1} = 128
columns of Wq/Wk/Wv and 128 rows of Wo. Each core computes Q^T/K^T/V for its
two heads over all tokens, runs attention for its 8 (batch, head) pairs, and
produces a partial output O_c = A_c @ Wo_c.  The all-reduce over the 8
partials is done on the host during unsharding.

On-chip layout (per batch b of 2048 tokens):
  - QKV^T tiles [128, 2048] (head-dim on partitions) from X^T resident chunks
  - V is PE-transposed back to token-major and augmented with a ones column,
    so the attention matmul accumulates both U^T = V^T P and the softmax
    denominators in one PSUM tile (row 64).
  - scores are computed transposed (S^T = K Q^T) with the two heads packed
    into disjoint PE row groups; one fused exp over both heads' PSUM banks.
  - no max-subtraction: scores ~ N(0,1) after the 1/sqrt(d) scale, |s| < ~7.

The softmax normalization is a 3-stage software pipeline over the global
(batch, q-tile) iteration index so the vector engine's strict-FIFO queue
never head-blocks on the DMA broadcast roundtrip (which stalled the PE for
~5.5us/iteration and re-throttled HAM):
  t:   U stop -> usb copy (PSUM->SBUF), d-rows -> DRAM, gather [128,8]
  t+1: reciprocal on [128,8] (batched: 32 slow [1,512] recips -> 16 tiny
       ones), scatter to DRAM, broadcast [64,512] per head
  t+2: aT = usb * bc muls, out-projection matmuls, output DMA (bf16)
"""
import os
import sys

for _p in ("/opt/trn_rl_repo", "/root/.axon_site/_ro/trn_rl_repo"):
    if os.path.isdir(_p) and _p not in sys.path:
        sys.path.append(_p)

from contextlib import ExitStack

import numpy as np
import ml_dtypes

import concourse.bass as bass
import concourse.tile as tile
from concourse import mybir
from concourse.bass_utils import run_bass_kernel_spmd
from concourse.masks import make_identity

BF16 = mybir.dt.bfloat16
F32 = mybir.dt.float32
EXP = mybir.ActivationFunctionType.Exp
NP_BF16 = ml_dtypes.bfloat16

B, S, D = 4, 2048, 1024
H, HD = 16, 64
N_CORES = 8
T = B * S  # 8192 tokens
KC = D // 128  # 8 contraction chunks
SCALE = 1.0 / np.sqrt(HD)

# ---------------------------------------------------------------------------
# Tile patches: this walrus build rejects instructions with more than one
# sync wait ("Too many sync wait commands"), so split extra waits into
# preceding same-engine nops, and replace the kernel-tail drain's wait list
# with a chain of single-wait SP nops.
# ---------------------------------------------------------------------------
_MAX_WAITS = 1
_patched = False


def _install_tile_patches():
    global _patched
    if _patched:
        return
    _patched = True
    from concourse.vector_clock import ScopedClock, VectorClock

    orig_lower = tile.TileContext._lower_ordered_insts

    def split_inst_waits(self, ordered):
        for bb_name in list(ordered.keys()):
            insts = ordered[bb_name]
            new = []
            for inst in insts:
                si = inst.sync_info
                if si is not None and len(si.on_wait) > _MAX_WAITS:
                    waits = list(si.on_wait)
                    head, tail = waits[:-_MAX_WAITS], waits[-_MAX_WAITS:]
                    for w in head:
                        nop = mybir.InstNoOp(
                            name=f"ws-{self.nc.next_id()}",
                            engine=inst.engine,
                            bass_nofuse=True,
                        )
                        nop.sync_info = mybir.SyncInfo(on_wait=[w], on_update=[])
                        new.append(nop)
                    inst.sync_info = mybir.SyncInfo(
                        on_wait=tail, on_update=list(si.on_update)
                    )
                new.append(inst)
            ordered[bb_name] = new
        return orig_lower(self, ordered)

    def split_drain_and_barrier(self, tick_clock, wait_clock):
        gc = tick_clock.global_clock
        ticks = eval(repr(gc).replace("VectorClock", ""))
        procs = [(i, t) for i, t in enumerate(ticks) if t > 0]
        for i in range(0, len(procs), _MAX_WAITS):
            chunk = procs[i : i + _MAX_WAITS]
            nop = self.nc.sync.nop(nofuse=True, hint="drain_wait_split")
            pc = VectorClock()
            for proc, tick in chunk:
                pc.require_at_least(proc, tick)
            wait_clock.add_sem_waits(nop.ins, ScopedClock({None: pc}))
        drain_inst = self.nc.sync.drain()
        wait_clock.add_sem_waits(
            drain_inst.ins, ScopedClock({None: gc}), ScopedClock({None: gc.copy()})
        )
        self.nc.all_engine_barrier()
        assert self.sems is not None
        popped = self.nc._tile_sem_poison_stack.pop()
        assert popped is self._sem_poison
        self.nc.clear_and_free_semaphores(list(self.sems.allocated().values()))
        self.nc.all_engine_barrier()

    tile.TileContext._lower_ordered_insts = split_inst_waits
    tile.TileContext._drain_and_barrier = split_drain_and_barrier


# ---------------------------------------------------------------------------
# Device kernel
# ---------------------------------------------------------------------------
def build_attention_nc(with_bias=True, probe=None, out_bf16=True):
    _install_tile_patches()
    nc = bass.Bass()

    xT = nc.declare_dram_parameter("xT", [KC, 128, T], BF16, isOutput=False)
    # weights partition-major [128, KC, 128] so the load DMA is contiguous
    # per partition (host pre-transposes)
    wq = nc.declare_dram_parameter("wq", [128, KC, 128], BF16, isOutput=False)
    wk = nc.declare_dram_parameter("wk", [128, KC, 128], BF16, isOutput=False)
    wv = nc.declare_dram_parameter("wv", [128, KC, 128], BF16, isOutput=False)
    if with_bias:
        bq = nc.declare_dram_parameter("bq", [128], BF16, isOutput=False)
        bk = nc.declare_dram_parameter("bk", [128], BF16, isOutput=False)
        bv = nc.declare_dram_parameter("bv", [128], BF16, isOutput=False)
    else:
        bq = bk = bv = None
    wo = nc.declare_dram_parameter("wo", [128, D], BF16, isOutput=False)
    out = nc.declare_dram_parameter(
        "out", [T, D], BF16 if out_bf16 else F32, isOutput=True
    )

    with tile.TileContext(nc) as tc, ExitStack() as ctx:
        singles = ctx.enter_context(tc.tile_pool(name="singles", bufs=1))
        px = ctx.enter_context(tc.tile_pool(name="px", bufs=16))
        pqk = ctx.enter_context(tc.tile_pool(name="pqk", bufs=2))
        pv = ctx.enter_context(tc.tile_pool(name="pv", bufs=2))
        pa = ctx.enter_context(tc.tile_pool(name="pa", bufs=3))
        ppt = ctx.enter_context(tc.tile_pool(name="ppt", bufs=4))
        pusb = ctx.enter_context(tc.tile_pool(name="pusb", bufs=8))
        pbc = ctx.enter_context(tc.tile_pool(name="pbc", bufs=6))
        pdg = ctx.enter_context(tc.tile_pool(name="pdg", bufs=4))
        pob = ctx.enter_context(tc.tile_pool(name="pob", bufs=3))
        dsc = ctx.enter_context(tc.tile_pool(name="dsc", bufs=8, space="DRAM"))
        psA = ctx.enter_context(tc.tile_pool(name="psA", bufs=2, space="PSUM"))
        psS = ctx.enter_context(tc.tile_pool(name="psS", bufs=2, space="PSUM"))
        psU = ctx.enter_context(tc.tile_pool(name="psU", bufs=2, space="PSUM"))

        # --- constants / weights, loaded once -----------------------------
        w_sb = {}
        b_sb = {}
        for name, wd, bd in (("q", wq, bq), ("k", wk, bk), ("v", wv, bv)):
            w_t = singles.tile([128, KC, 128], BF16, tag=f"w{name}")
            nc.sync.dma_start(w_t, wd[:, :, :])
            w_sb[name] = w_t
            if with_bias:
                b_t = singles.tile([1, 128], BF16, tag=f"b{name}")
                nc.sync.dma_start(b_t, bd[:][None, :])
                b_sb[name] = b_t
        wo_sb = singles.tile([128, D], BF16, tag="wo")
        nc.sync.dma_start(wo_sb, wo[:, :])
        if with_bias:
            ones_sb = singles.tile([1, 512], BF16, tag="ones")
            nc.vector.memset(ones_sb, 1.0)
        ident = singles.tile([128, 128], BF16, tag="ident")
        make_identity(nc, ident)

        # per-batch state created by the A-slices
        state = [dict() for _ in range(B)]
        # per-(b,qt) normalization pipeline state, keyed by global iter idx
        norm = {}

        def a_slice(b, qt):
            """Emit 1/4 of batch b's QKV projections (+ V transposes)."""
            st = state[b]
            if qt == 0:
                st["x"] = [
                    px.tile([128, S], BF16, tag="x", name=f"x_{b}_{kc}")
                    for kc in range(KC)
                ]
                for kc in range(KC):
                    nc.sync.dma_start(
                        st["x"][kc], xT[kc, :, b * S : (b + 1) * S]
                    )
                for name in ("q", "k", "v"):
                    pool = pqk if name != "v" else pv
                    st[name] = pool.tile([128, S], BF16, tag=f"{name}T", name=f"{name}T_{b}")
                st["vS0"] = pv.tile([128, 16, 65], BF16, tag="vS0", name=f"vS0_{b}")
                st["vS1"] = pv.tile([128, 16, 65], BF16, tag="vS1", name=f"vS1_{b}")
                nc.vector.memset(st["vS0"][:, :, 64:65], 1.0)
                nc.vector.memset(st["vS1"][:, :, 64:65], 1.0)
            for name in ("q", "k", "v"):
                dst = st[name]
                w_t, b_t = w_sb[name], b_sb.get(name)
                ps = psA.tile([128, 512], F32, tag="psA")
                for kc in range(KC):
                    nc.tensor.matmul(
                        ps,
                        w_t[:, kc, :],
                        st["x"][kc][:, qt * 512 : (qt + 1) * 512],
                        start=(kc == 0),
                        stop=(not with_bias and kc == KC - 1),
                    )
                if with_bias:
                    nc.tensor.matmul(ps, b_t, ones_sb, start=False, stop=True)
                nc.vector.tensor_copy(dst[:, qt * 512 : (qt + 1) * 512], ps)
            # transpose this quarter of V into token-major + ones layout
            for t in range(qt * 4, qt * 4 + 4):
                tp = psU.tile([128, 128], BF16, tag="u")
                nc.tensor.transpose(tp, st["v"][:, t * 128 : (t + 1) * 128], ident)
                nc.vector.tensor_copy(st["vS0"][:, t, 0:64], tp[:, 0:64])
                nc.vector.tensor_copy(st["vS1"][:, t, 0:64], tp[:, 64:128])

        def b_block(b, qt, it):
            """scores^T -> exp -> attention, for one q-tile; stage-0 of the
            normalization pipeline (usb copies + d rows to DRAM + gather)."""
            st = state[b]
            qT, kT = st["q"], st["k"]
            q0, q1 = qt * 512, (qt + 1) * 512
            u0 = psU.tile([128, 512], F32, tag="u")
            u1 = psU.tile([128, 512], F32, tag="u")
            for kc in range(16):
                k0 = kc * 128
                sp = psS.tile([128, 1024], F32, tag="psS")
                nc.tensor.matmul(
                    sp[:, 0:512], kT[0:64, k0 : k0 + 128], qT[0:64, q0:q1],
                    start=True, stop=True, tile_position=(0, 0),
                )
                nc.tensor.matmul(
                    sp[:, 512:1024], kT[64:128, k0 : k0 + 128], qT[64:128, q0:q1],
                    start=True, stop=True, tile_position=(64, 0),
                )
                pt = ppt.tile([128, 1024], BF16, tag="pt")
                nc.scalar.activation(pt, sp, EXP, scale=float(SCALE))
                nc.tensor.matmul(
                    u0[0:65, :], st["vS0"][:, kc, :], pt[:, 0:512],
                    start=(kc == 0), stop=(kc == 15),
                )
                nc.tensor.matmul(
                    u1[0:65, :], st["vS1"][:, kc, :], pt[:, 512:1024],
                    start=(kc == 0), stop=(kc == 15),
                )
            nst = {}
            # copy out of PSUM right away so the u slots free for the next
            # q-tile; the norm chain continues on SBUF tiles.
            for h, u in ((0, u0), (1, u1)):
                usb = pusb.tile([65, 512], F32, tag="usb")
                nc.vector.tensor_copy(usb, u[0:65, :])
                nst[f"usb{h}"] = usb
            d2 = dsc.tile([2, 512], F32, tag="d2")
            nc.sync.dma_start(d2[0:1, :], nst["usb0"][64:65, :])
            nc.sync.dma_start(d2[1:2, :], nst["usb1"][64:65, :])
            dg = pdg.tile([128, 8], F32, tag="dg")
            nc.sync.dma_start(dg, d2[:, :].rearrange("a (x c) -> (a x) c", x=64))
            nst["dg"] = dg
            norm[it] = nst

        def norm_back(it):
            """Stage-1: batched reciprocal + scatter + per-head broadcast."""
            nst = norm[it]
            rg = pdg.tile([128, 8], F32, tag="rg")
            nc.vector.reciprocal(rg, nst["dg"])
            di2 = dsc.tile([2, 512], F32, tag="di2")
            nc.sync.dma_start(
                di2[:, :].rearrange("a (x c) -> (a x) c", x=64), rg
            )
            for h in range(2):
                bc = pbc.tile([64, 512], F32, tag="bc")
                nc.sync.dma_start(bc, di2[h : h + 1, :].to_broadcast((64, 512)))
                nst[f"bc{h}"] = bc

        def c_slice(b, j, it):
            """Stage-2: normalize aT then emit 4 output-projection token
            tiles for batch b (tt=4j..4j+3)."""
            nst = norm.pop(it)
            aTq = pa.tile([128, 512], BF16, tag="aT", name=f"aT_{b}_{j}")
            for h in range(2):
                nc.vector.tensor_mul(
                    aTq[h * 64 : (h + 1) * 64, :],
                    nst[f"usb{h}"][0:64, :],
                    nst[f"bc{h}"],
                )
            for tt in range(4 * j, 4 * j + 4):
                col = (tt - 4 * j) * 128
                ob = pob.tile([128, 1024], BF16 if out_bf16 else F32, tag="ob")
                for g in range(2):
                    po = psA.tile([128, 512], F32, tag="psA")
                    nc.tensor.matmul(
                        po,
                        aTq[:, col : col + 128],
                        wo_sb[:, g * 512 : (g + 1) * 512],
                        start=True,
                        stop=True,
                    )
                    nc.vector.tensor_copy(ob[:, g * 512 : (g + 1) * 512], po)
                nc.sync.dma_start(
                    out[b * S + tt * 128 : b * S + (tt + 1) * 128, :], ob
                )

        # software pipeline over global iteration t = 4*b + qt:
        #   b_block(t) leads; a_slice(t+4) and the trailing norm_back(t-1) /
        #   c_slice(t-2) stages fill engine gaps.
        for qt in range(4):
            a_slice(0, qt)
        for t in range(B * 4):
            b, qt = divmod(t, 4)
            b_block(b, qt, t)
            if b + 1 < B:
                a_slice(b + 1, qt)
            if t >= 1:
                norm_back(t - 1)
            if t >= 2:
                c_slice((t - 2) // 4, (t - 2) % 4, t - 2)
        norm_back(B * 4 - 1)
        c_slice(B - 1, 2, B * 4 - 2)
        c_slice(B - 1, 3, B * 4 - 1)

    return nc


_NC_CACHE = {}


def _get_nc(with_bias=True, probe=None, out_bf16=True):
    key = (with_bias, probe, out_bf16)
    if key not in _NC_CACHE:
        _NC_CACHE[key] = build_attention_nc(with_bias, probe, out_bf16)
    return _NC_CACHE[key]


def _run(inputs, Wq, bq, Wk, bk, Wv, bv, Wo, bo, trace=False, **spmd_kwargs):
    X2 = np.asarray(inputs, dtype=np.float32).reshape(T, D)
    xT = X2.T.astype(NP_BF16).reshape(KC, 128, T)
    with_bias = bool(
        np.any(np.asarray(bq)) or np.any(np.asarray(bk)) or np.any(np.asarray(bv))
    )

    def wprep(W, cs):
        # [D, 128] -> [KC, 128, 128] -> partition-major [128, KC, 128]
        return np.ascontiguousarray(
            np.asarray(W[:, cs]).reshape(KC, 128, 128).transpose(1, 0, 2)
        ).astype(NP_BF16)

    in_maps = []
    for c in range(N_CORES):
        cs = slice(c * 128, (c + 1) * 128)
        in_maps.append(
            {
                "xT": xT,
                "wq": wprep(Wq, cs),
                "wk": wprep(Wk, cs),
                "wv": wprep(Wv, cs),
                "bq": np.asarray(bq[cs]).astype(NP_BF16),
                "bk": np.asarray(bk[cs]).astype(NP_BF16),
                "bv": np.asarray(bv[cs]).astype(NP_BF16),
                "wo": np.ascontiguousarray(Wo[cs, :]).astype(NP_BF16),
            }
        )

    if not with_bias:
        for m in in_maps:
            m.pop("bq"), m.pop("bk"), m.pop("bv")
    res = run_bass_kernel_spmd(
        _get_nc(with_bias), in_maps, list(range(N_CORES)), trace=trace, **spmd_kwargs
    )
    acc = res.results[0]["out"].astype(np.float32)
    for c in range(1, N_CORES):
        acc += res.results[c]["out"]
    acc += np.asarray(bo, dtype=np.float32)[None, :]
    return acc.reshape(B, S, D), res


def kernel(inputs, Wq, bq, Wk, bk, Wv, bv, Wo, bo):
    out, _ = _run(inputs, Wq, bq, Wk, bk, Wv, bv, Wo, bo)
    return out
